# revision 17
# baseline (speedup 1.0000x reference)
"""Trainium2 Bass kernel for (W0 (x) W1 (x) W2 (x) W3) @ x  -- Kronecker chain.

Shapes: x [2^20, 32] fp32, Wi [32, 32] fp32. Output [2^20, 32] fp32.

Strategy (8 NeuronCores, batch-sharded: core c owns x[:, 4c:4c+4]):
View x_c as t[j0,j1,j2,j3,b] = [32,32,32,32,4]. Four 32-contractions.

PE-crossing architecture: stages 1-3 run the matmul with the DATA as the
stationary operand and an augmented 128x128 weight (delta-structured
W (x) I4 permutation) as the moving operand, so the output partitions
become the chunk's columns -- the NEXT contraction mode rotates onto the
partition axis inside the matmul itself. Stage 4 is a normal
weight-stationary matmul whose output partition order (i0*4 + i3b) makes
the store contiguous.

Stage order: j2, j3, j1, j0. Phase I streams 32 chunks over j0 (load +
S1 + S2 -> B1 bf16 resident); Phase II streams 8 chunks over i2a
(S3 + S4 + store). Host pre-shuffles x (bf16 cast) and post-shuffles y.

Schedule: both phases are software-pipelined at 4-matmul micro-step
granularity. Each stage cycles four [128,512] PSUM tiles (1 bank each);
the PE program order interleaves stage A of micro-step u with stage B of
micro-step u-SKEW so every PSUM->SBUF evacuation has several micro-steps
to complete before its result or its bank is needed. Evacuations are
assigned greedily (least-loaded, exact cost model) across Act/DVE/Pool.
"""
import numpy as np
import ml_dtypes

import concourse.bass as bass
import concourse.bacc as bacc
import concourse.mybir as mybir
import concourse.tile as tile
from concourse.bass_utils import run_bass_kernel_spmd

F32 = mybir.dt.float32
BF16 = mybir.dt.bfloat16

L = 32
N = L ** 4          # 1048576
B = 32
NCORES = 8
BC = B // NCORES    # 4
SKEW = 4            # micro-steps between a stage's output and its consumer

_NC_CACHE = {}
DESC = {}           # instruction name -> human label (for trace analysis)


def _build_nc():
    nc = bacc.Bacc("TRN2", target_bir_lowering=False, debug=False)

    def lab(inst, s):
        try:
            DESC[inst.ins.name] = s
        except Exception:
            pass
        return inst

    # x pre-shuffled on host to [j0, (j1h, j2), (j1l, j3, b)], bf16
    x = nc.dram_tensor("x", [32, 131072], BF16, kind="ExternalInput")
    w2a = nc.dram_tensor("w2a", [128, 128], BF16, kind="ExternalInput")
    w3a = nc.dram_tensor("w3a", [128, 128], BF16, kind="ExternalInput")
    w1a = nc.dram_tensor("w1a", [128, 128], BF16, kind="ExternalInput")
    w0a = nc.dram_tensor("w0a", [128, 128], BF16, kind="ExternalInput")
    # y device order: [i2a(8), (i0, i3b)(128), (i3a, b, i1, i2b)(4096)] fp32
    y = nc.dram_tensor("y", [8, 524288], BF16, kind="ExternalOutput")

    # Greedy least-loaded PSUM->SBUF evacuation across Act/DVE/Pool.
    # Costs mirror the TRN2 cost model (ns): Act (rows+222)/1.2,
    # DVE (rows+120)/0.96, Pool 95 + rows/(1.2*0.6).
    busy = {"scalar": 0.0, "vector": 0.0, "gpsimd": 0.0}

    def _evac_cost(eng, rows):
        if eng == "scalar":
            return (rows + 222) / 1.2
        if eng == "vector":
            return (rows + 120) / 0.96
        return 95.0 + rows / 0.72

    def evac(out_ap, in_ap, rows, tag, force=None, pool_ok=True):
        # Consumer-critical evacs (S1->t1, S3->t3 feed the PE within SKEW
        # micro-steps) avoid the slow Pool engine; S2/S4 evacs absorb it.
        cands = busy if pool_ok else {k: busy[k] for k in ("scalar", "vector")}
        eng = force or min(cands, key=lambda e: busy[e] + _evac_cost(e, rows))
        busy[eng] += _evac_cost(eng, rows)
        if eng == "scalar":
            r = nc.scalar.copy(out=out_ap, in_=in_ap)
        elif eng == "vector":
            r = nc.vector.tensor_copy(out_ap, in_ap)
        else:
            r = nc.gpsimd.tensor_copy(out_ap, in_ap)
        lab(r, f"evac:{tag}:{eng}")

    with tile.TileContext(nc) as tc:
        with tc.tile_pool(name="wp", bufs=1) as wp, \
             tc.tile_pool(name="b1p", bufs=1) as b1p:
            w2s = wp.tile([128, 128], BF16, name="w2s")
            w3s = wp.tile([128, 128], BF16, name="w3s")
            w1s = wp.tile([128, 128], BF16, name="w1s")
            w0s = wp.tile([128, 128], BF16, name="w0s")

            # B1: [part (i2b,j1), addr = i2a*4096 + i3a*512 + b*128 + i3b*32 + j0]
            b1 = b1p.tile([128, 32768], BF16, name="b1")
            b1_t, b1_o = b1.tensor, b1.offset

            # ---- Phase I: S1 (contract j2) + S2 (contract j3) ----
            # micro-step u = 2*j0 + h, h in {0,1}: S1 does j1l in 4h..4h+3,
            # S2 (at u-SKEW) does i2a in 4h..4h+3.
            with tc.tile_pool(name="lp", bufs=6) as lp, \
                 tc.tile_pool(name="t1p", bufs=6) as t1p, \
                 tc.tile_pool(name="ps1p", bufs=4, space="PSUM") as ps1p, \
                 tc.tile_pool(name="ps2p", bufs=4, space="PSUM") as ps2p:

                lts = {}
                t1s = {}

                def load_chunk(c, half=None):
                    if c in lts:
                        lt_t, lt_o = lts[c]
                    else:
                        ltile = lp.tile([128, 1024], BF16, name="lt")
                        lts[c] = (lt_t, lt_o) = (ltile.tensor, ltile.offset)
                    off, n = (0, 1024) if half is None else (half * 512, 512)
                    lab(nc.sync.dma_start(
                        out=bass.AP(lt_t, lt_o + off, [[1024, 128], [1, n]]),
                        in_=bass.AP(x, c * 131072 + off,
                                    [[1024, 128], [1, n]])),
                        f"load:c{c}.{half}")

                # startup: first chunk + w2 first so S1 starts ASAP;
                # the other weights trickle in behind the early loads.
                load_chunk(0)
                lab(nc.sync.dma_start(out=w2s[:], in_=w2a.ap()), "load:w2")
                load_chunk(1)
                lab(nc.sync.dma_start(out=w3s[:], in_=w3a.ap()), "load:w3")
                load_chunk(2)
                lab(nc.sync.dma_start(out=w1s[:], in_=w1a.ap()), "load:w1")
                load_chunk(3)
                lab(nc.sync.dma_start(out=w0s[:], in_=w0a.ap()), "load:w0")
                load_chunk(4)

                for u in range(64 + SKEW):
                    if u < 64:
                        c, h = u // 2, u % 2
                        if h == 0:
                            if c + 5 < 32:
                                load_chunk(c + 5)
                            t1 = t1p.tile([128, 1024], BF16, name="t1")
                            t1s[c] = (t1.tensor, t1.offset)
                        lt_t, lt_o = lts[c]
                        t1_t, t1_o = t1s[c]
                        p1 = ps1p.tile([128, 512], F32, name="p1")
                        for d in range(4):
                            j1l = 4 * h + d
                            lhsT = bass.AP(lt_t, lt_o + j1l * 128,
                                           [[1024, 128], [1, 128]])
                            lab(nc.tensor.matmul(
                                p1[:, d * 128:(d + 1) * 128],
                                lhsT, w2s[:], start=True, stop=True),
                                f"S1:u{u}.d{d}")
                        # psum cols (j1l, i2a, i2b, j1h) -> t1 addr
                        # i2a*128 + i2b*32 + j1h*8 + j1l
                        evac(bass.AP(t1_t, t1_o + 4 * h,
                                     [[1024, 128], [1, 4], [128, 8], [8, 16]]),
                             p1[:], 512, f"S1:u{u}", pool_ok=False)

                    if u >= SKEW:
                        u2 = u - SKEW
                        c2, h2 = u2 // 2, u2 % 2
                        t1_t, t1_o = t1s[c2]
                        p2 = ps2p.tile([128, 512], F32, name="p2")
                        for d in range(4):
                            i2a = 4 * h2 + d
                            lhsT = bass.AP(t1_t, t1_o + i2a * 128,
                                           [[1024, 128], [1, 128]])
                            lab(nc.tensor.matmul(
                                p2[:, d * 128:(d + 1) * 128],
                                lhsT, w3s[:], start=True, stop=True),
                                f"S2:u{u2}.d{d}")
                        # psum cols (i2a, i3a, b, i3b) -> b1 addr
                        # i2a*4096 + i3a*512 + b*128 + i3b*32 + j0
                        evac(bass.AP(b1_t, b1_o + c2 + 4 * h2 * 4096,
                                     [[32768, 128], [4096, 4], [512, 8],
                                      [32, 16]]),
                             p2[:], 512, f"S2:u{u2}",
                             force="scalar" if u2 >= 62 else None)

            # ---- Phase II: S3 (contract j1) + S4 (contract j0) ----
            # micro-step v = 8*k + g: S3 does cq in 4g..4g+3 (cq=i3a*4+b),
            # S4 (at v-SKEW) does i3a = g.
            with tc.tile_pool(name="t3p", bufs=3) as t3p, \
                 tc.tile_pool(name="stgp", bufs=3) as stgp, \
                 tc.tile_pool(name="ps3p", bufs=4, space="PSUM") as ps3p, \
                 tc.tile_pool(name="ps4p", bufs=4, space="PSUM") as ps4p:

                t3s = {}
                stgs = {}
                for v in range(64 + SKEW):
                    if v < 64:
                        k, g = v // 8, v % 8
                        if g == 0:
                            t3 = t3p.tile([128, 4096], BF16, name="t3")
                            t3s[k] = (t3.tensor, t3.offset)
                            stg = stgp.tile([128, 4096], BF16, name="stg")
                            stgs[k] = (stg.tensor, stg.offset)
                        t3_t, t3_o = t3s[k]
                        p3 = ps3p.tile([128, 512], F32, name="p3")
                        for d in range(4):
                            cq = 4 * g + d
                            lhsT = bass.AP(b1_t, b1_o + k * 4096 + cq * 128,
                                           [[32768, 128], [1, 128]])
                            lab(nc.tensor.matmul(
                                p3[:, d * 128:(d + 1) * 128],
                                lhsT, w1s[:], start=True, stop=True),
                                f"S3:v{v}.d{d}")
                        # psum cols map flat into t3: addr = i3a*512+b*128+n
                        evac(bass.AP(t3_t, t3_o + g * 512,
                                     [[4096, 128], [1, 512]]),
                             p3[:], 512, f"S3:v{v}", pool_ok=False)

                    if v >= SKEW:
                        v2 = v - SKEW
                        k2, g2 = v2 // 8, v2 % 8
                        t3_t, t3_o = t3s[k2]
                        stg_t, stg_o = stgs[k2]
                        p4 = ps4p.tile([128, 512], F32, name="p4")
                        rhs = bass.AP(t3_t, t3_o + g2 * 512,
                                      [[4096, 128], [128, 4], [1, 128]])
                        lab(nc.tensor.matmul(p4[:], w0s[:], rhs,
                                             start=True, stop=True),
                            f"S4:v{v2}")
                        evac(bass.AP(stg_t, stg_o + g2 * 512,
                                     [[4096, 128], [1, 512]]),
                             p4[:], 512, f"S4:v{v2}",
                             force="scalar" if v2 >= 62 else None)
                        if k2 < 7:
                            if g2 == 7:
                                lab(nc.sync.dma_start(
                                    out=bass.AP(y, k2 * 524288,
                                                [[4096, 128], [1, 4096]]),
                                    in_=bass.AP(stg_t, stg_o,
                                                [[4096, 128], [1, 4096]])),
                                    f"store:k{k2}")
                        else:
                            # tail: stream the final chunk out in 512-slices
                            lab(nc.sync.dma_start(
                                out=bass.AP(y, k2 * 524288 + g2 * 512,
                                            [[4096, 128], [1, 512]]),
                                in_=bass.AP(stg_t, stg_o + g2 * 512,
                                            [[4096, 128], [1, 512]])),
                                f"store:k{k2}.{g2}")

    nc.finalize()
    return nc


def _build_waug(w: np.ndarray, kind: str) -> np.ndarray:
    """Augmented 128x128 weights (see apsim2.py)."""
    wa = np.zeros((128, 128), dtype=np.float32)
    ar = np.arange(32)
    if kind == "w3":
        # rows p = j3*4 + b ; cols n = i3a*16 + b*4 + i3b
        for b in range(4):
            cols = (ar >> 2) * 16 + b * 4 + (ar & 3)
            wa[np.ix_(ar * 4 + b, cols)] = w.T
    else:
        # rows p = q*32 + j ; cols n = i*4 + q
        for q in range(4):
            wa[np.ix_(q * 32 + ar, ar * 4 + q)] = w.T
    return wa


def _get_nc():
    if "nc" not in _NC_CACHE:
        _NC_CACHE["nc"] = _build_nc()
    return _NC_CACHE["nc"]


def make_in_maps(x, W0, W1, W2, W3):
    x = np.asarray(x, dtype=np.float32)
    bf = ml_dtypes.bfloat16
    w2a = _build_waug(np.asarray(W2, np.float32), "q").astype(bf)
    w3a = _build_waug(np.asarray(W3, np.float32), "w3").astype(bf)
    w1a = _build_waug(np.asarray(W1, np.float32), "q").astype(bf)
    w0a = _build_waug(np.asarray(W0, np.float32), "q").astype(bf)
    xr = x.reshape(32, 4, 8, 32, 32, B)
    in_maps = []
    for c in range(NCORES):
        xc = xr[..., c * BC:(c + 1) * BC].transpose(0, 1, 3, 2, 4, 5)
        xc = np.ascontiguousarray(xc).astype(bf).reshape(32, 131072)
        in_maps.append({"x": xc, "w2a": w2a, "w3a": w3a,
                        "w1a": w1a, "w0a": w0a})
    return in_maps


def _unshuffle_y(yd: np.ndarray) -> np.ndarray:
    """[i2a(8), (i0, i3b), (i3a, b, i1, i2b)] -> [N, BC]."""
    y = yd.astype(np.float32).reshape(8, 32, 4, 8, BC, 32, 4)
    y = y.transpose(1, 5, 0, 6, 3, 2, 4)
    return np.ascontiguousarray(y).reshape(N, BC)


def kernel(x, W0, W1, W2, W3, _trace=False):
    nc = _get_nc()
    in_maps = make_in_maps(x, W0, W1, W2, W3)
    res = run_bass_kernel_spmd(nc, in_maps, core_ids=list(range(NCORES)),
                               trace=_trace)
    out = np.concatenate(
        [_unshuffle_y(res.results[c]["y"]) for c in range(NCORES)], axis=1)
    if _trace:
        kernel.last_result = res
    return out


if __name__ == "__main__":
    rng = np.random.default_rng(0)
    x = rng.standard_normal((N, B), dtype=np.float32)
    ws = [rng.standard_normal((L, L), dtype=np.float32) for _ in range(4)]
    y = kernel(x, *ws)
    print("ran", y.shape, y.dtype)


# revision 18
# speedup vs baseline: 1.0182x; 1.0182x over previous
"""Trainium2 Bass kernel for (W0 (x) W1 (x) W2 (x) W3) @ x  -- Kronecker chain.

Shapes: x [2^20, 32] fp32, Wi [32, 32] fp32. Output [2^20, 32] fp32.

Strategy (8 NeuronCores, batch-sharded: core c owns x[:, 4c:4c+4]):
View x_c as t[j0,j1,j2,j3,b] = [32,32,32,32,4]. Four 32-contractions.

PE-crossing architecture: stages 1-3 run the matmul with the DATA as the
stationary operand and an augmented 128x128 weight (delta-structured
W (x) I4 permutation) as the moving operand, so the output partitions
become the chunk's columns -- the NEXT contraction mode rotates onto the
partition axis inside the matmul itself. Stage 4 is a normal
weight-stationary matmul whose output partition order (i0*4 + i3b) makes
the store contiguous.

Stage order: j2, j3, j1, j0. Phase I streams 32 chunks over j0 (load +
S1 + S2 -> B1 bf16 resident); Phase II streams 8 chunks over i2a
(S3 + S4 + store). Host pre-shuffles x (bf16 cast) and post-shuffles y.

Schedule: both phases are software-pipelined at 4-matmul micro-step
granularity. Each stage cycles four [128,512] PSUM tiles (1 bank each);
the PE program order interleaves stage A of micro-step u with stage B of
micro-step u-SKEW so every PSUM->SBUF evacuation has several micro-steps
to complete before its result or its bank is needed. Evacuations are
assigned greedily (least-loaded, exact cost model) across Act/DVE/Pool.
"""
import numpy as np
import ml_dtypes

import concourse.bass as bass
import concourse.bacc as bacc
import concourse.mybir as mybir
import concourse.tile as tile
from concourse.bass_utils import run_bass_kernel_spmd

F32 = mybir.dt.float32
BF16 = mybir.dt.bfloat16

L = 32
N = L ** 4          # 1048576
B = 32
NCORES = 8
BC = B // NCORES    # 4
SKEW = 4            # micro-steps between a stage's output and its consumer

_NC_CACHE = {}
DESC = {}           # instruction name -> human label (for trace analysis)


def _build_nc():
    nc = bacc.Bacc("TRN2", target_bir_lowering=False, debug=False)

    def lab(inst, s):
        try:
            DESC[inst.ins.name] = s
        except Exception:
            pass
        return inst

    # x pre-shuffled on host to [j0, (j1h, j2), (j1l, j3, b)], bf16
    x = nc.dram_tensor("x", [32, 131072], BF16, kind="ExternalInput")
    w2a = nc.dram_tensor("w2a", [128, 128], BF16, kind="ExternalInput")
    w3a = nc.dram_tensor("w3a", [128, 128], BF16, kind="ExternalInput")
    w1a = nc.dram_tensor("w1a", [128, 128], BF16, kind="ExternalInput")
    w0a = nc.dram_tensor("w0a", [128, 128], BF16, kind="ExternalInput")
    # y device order: [i2a(8), (i0, i3b)(128), (i3a, b, i1, i2b)(4096)] fp32
    y = nc.dram_tensor("y", [8, 524288], BF16, kind="ExternalOutput")

    # Greedy least-loaded PSUM->SBUF evacuation across Act/DVE/Pool.
    # Costs mirror the TRN2 cost model (ns): Act (rows+222)/1.2,
    # DVE (rows+120)/0.96, Pool 95 + rows/(1.2*0.6).
    busy = {"scalar": 0.0, "vector": 0.0, "gpsimd": 0.0}

    def _evac_cost(eng, rows):
        if eng == "scalar":
            return (rows + 222) / 1.2
        if eng == "vector":
            return (rows + 120) / 0.96
        return 95.0 + rows / 0.72

    def evac(out_ap, in_ap, rows, tag, force=None, pool_ok=True):
        # Consumer-critical evacs (S1->t1, S3->t3 feed the PE within SKEW
        # micro-steps) avoid the slow Pool engine; S2/S4 evacs absorb it.
        cands = busy if pool_ok else {k: busy[k] for k in ("scalar", "vector")}
        eng = force or min(cands, key=lambda e: busy[e] + _evac_cost(e, rows))
        busy[eng] += _evac_cost(eng, rows)
        if eng == "scalar":
            r = nc.scalar.copy(out=out_ap, in_=in_ap)
        elif eng == "vector":
            r = nc.vector.tensor_copy(out_ap, in_ap)
        else:
            r = nc.gpsimd.tensor_copy(out_ap, in_ap)
        lab(r, f"evac:{tag}:{eng}")

    with tile.TileContext(nc) as tc:
        with tc.tile_pool(name="wp", bufs=1) as wp, \
             tc.tile_pool(name="b1p", bufs=1) as b1p:
            w2s = wp.tile([128, 128], BF16, name="w2s")
            w3s = wp.tile([128, 128], BF16, name="w3s")
            w1s = wp.tile([128, 128], BF16, name="w1s")
            w0s = wp.tile([128, 128], BF16, name="w0s")

            # B1: [part (i2b,j1), addr = i2a*4096 + i3a*512 + b*128 + i3b*32 + j0]
            b1 = b1p.tile([128, 32768], BF16, name="b1")
            b1_t, b1_o = b1.tensor, b1.offset

            # ---- Phase I: S1 (contract j2) + S2 (contract j3) ----
            # micro-step u = 2*j0 + h, h in {0,1}: S1 does j1l in 4h..4h+3,
            # S2 (at u-SKEW) does i2a in 4h..4h+3.
            with tc.tile_pool(name="lp", bufs=6) as lp, \
                 tc.tile_pool(name="t1p", bufs=6) as t1p, \
                 tc.tile_pool(name="ps1p", bufs=4, space="PSUM") as ps1p, \
                 tc.tile_pool(name="ps2p", bufs=4, space="PSUM") as ps2p:

                lts = {}
                t1s = {}

                def load_chunk(c, half=None):
                    if c in lts:
                        lt_t, lt_o = lts[c]
                    else:
                        ltile = lp.tile([128, 1024], BF16, name="lt")
                        lts[c] = (lt_t, lt_o) = (ltile.tensor, ltile.offset)
                    off, n = (0, 1024) if half is None else (half * 512, 512)
                    lab(nc.sync.dma_start(
                        out=bass.AP(lt_t, lt_o + off, [[1024, 128], [1, n]]),
                        in_=bass.AP(x, c * 131072 + off,
                                    [[1024, 128], [1, n]])),
                        f"load:c{c}.{half}")

                # startup: first chunk + w2 first so S1 starts ASAP;
                # the other weights trickle in behind the early loads.
                load_chunk(0)
                lab(nc.sync.dma_start(out=w2s[:], in_=w2a.ap()), "load:w2")
                lab(nc.sync.dma_start(out=w3s[:], in_=w3a.ap()), "load:w3")
                load_chunk(1)
                lab(nc.sync.dma_start(out=w1s[:], in_=w1a.ap()), "load:w1")
                load_chunk(2)
                lab(nc.sync.dma_start(out=w0s[:], in_=w0a.ap()), "load:w0")
                load_chunk(3)
                load_chunk(4)

                for u in range(64 + SKEW):
                    if u < 64:
                        c, h = u // 2, u % 2
                        if h == 0:
                            if c + 5 < 32:
                                load_chunk(c + 5)
                            t1 = t1p.tile([128, 1024], BF16, name="t1")
                            t1s[c] = (t1.tensor, t1.offset)
                        lt_t, lt_o = lts[c]
                        t1_t, t1_o = t1s[c]
                        p1 = ps1p.tile([128, 512], F32, name="p1")
                        for d in range(4):
                            j1l = 4 * h + d
                            lhsT = bass.AP(lt_t, lt_o + j1l * 128,
                                           [[1024, 128], [1, 128]])
                            lab(nc.tensor.matmul(
                                p1[:, d * 128:(d + 1) * 128],
                                lhsT, w2s[:], start=True, stop=True),
                                f"S1:u{u}.d{d}")
                        # psum cols (j1l, i2a, i2b, j1h) -> t1 addr
                        # i2a*128 + i2b*32 + j1h*8 + j1l
                        evac(bass.AP(t1_t, t1_o + 4 * h,
                                     [[1024, 128], [1, 4], [128, 8], [8, 16]]),
                             p1[:], 512, f"S1:u{u}", pool_ok=False)

                    if u >= SKEW:
                        u2 = u - SKEW
                        c2, h2 = u2 // 2, u2 % 2
                        t1_t, t1_o = t1s[c2]
                        p2 = ps2p.tile([128, 512], F32, name="p2")
                        for d in range(4):
                            i2a = 4 * h2 + d
                            lhsT = bass.AP(t1_t, t1_o + i2a * 128,
                                           [[1024, 128], [1, 128]])
                            lab(nc.tensor.matmul(
                                p2[:, d * 128:(d + 1) * 128],
                                lhsT, w3s[:], start=True, stop=True),
                                f"S2:u{u2}.d{d}")
                        # psum cols (i2a, i3a, b, i3b) -> b1 addr
                        # i2a*4096 + i3a*512 + b*128 + i3b*32 + j0
                        evac(bass.AP(b1_t, b1_o + c2 + 4 * h2 * 4096,
                                     [[32768, 128], [4096, 4], [512, 8],
                                      [32, 16]]),
                             p2[:], 512, f"S2:u{u2}")

            # ---- Phase II: S3 (contract j1) + S4 (contract j0) ----
            # micro-step v = 8*k + g: S3 does cq in 4g..4g+3 (cq=i3a*4+b),
            # S4 (at v-SKEW) does i3a = g.
            with tc.tile_pool(name="t3p", bufs=3) as t3p, \
                 tc.tile_pool(name="stgp", bufs=3) as stgp, \
                 tc.tile_pool(name="ps3p", bufs=4, space="PSUM") as ps3p, \
                 tc.tile_pool(name="ps4p", bufs=4, space="PSUM") as ps4p:

                t3s = {}
                stgs = {}
                for v in range(64 + SKEW):
                    if v < 64:
                        k, g = v // 8, v % 8
                        if g == 0:
                            t3 = t3p.tile([128, 4096], BF16, name="t3")
                            t3s[k] = (t3.tensor, t3.offset)
                            stg = stgp.tile([128, 4096], BF16, name="stg")
                            stgs[k] = (stg.tensor, stg.offset)
                        t3_t, t3_o = t3s[k]
                        p3 = ps3p.tile([128, 512], F32, name="p3")
                        for d in range(4):
                            cq = 4 * g + d
                            lhsT = bass.AP(b1_t, b1_o + k * 4096 + cq * 128,
                                           [[32768, 128], [1, 128]])
                            lab(nc.tensor.matmul(
                                p3[:, d * 128:(d + 1) * 128],
                                lhsT, w1s[:], start=True, stop=True),
                                f"S3:v{v}.d{d}")
                        # psum cols map flat into t3: addr = i3a*512+b*128+n
                        evac(bass.AP(t3_t, t3_o + g * 512,
                                     [[4096, 128], [1, 512]]),
                             p3[:], 512, f"S3:v{v}", pool_ok=False)

                    if v >= SKEW:
                        v2 = v - SKEW
                        k2, g2 = v2 // 8, v2 % 8
                        t3_t, t3_o = t3s[k2]
                        stg_t, stg_o = stgs[k2]
                        p4 = ps4p.tile([128, 512], F32, name="p4")
                        rhs = bass.AP(t3_t, t3_o + g2 * 512,
                                      [[4096, 128], [128, 4], [1, 128]])
                        lab(nc.tensor.matmul(p4[:], w0s[:], rhs,
                                             start=True, stop=True),
                            f"S4:v{v2}")
                        evac(bass.AP(stg_t, stg_o + g2 * 512,
                                     [[4096, 128], [1, 512]]),
                             p4[:], 512, f"S4:v{v2}",
                             force="scalar" if v2 >= 62 else None)
                        if k2 < 7:
                            if g2 == 7:
                                lab(nc.sync.dma_start(
                                    out=bass.AP(y, k2 * 524288,
                                                [[4096, 128], [1, 4096]]),
                                    in_=bass.AP(stg_t, stg_o,
                                                [[4096, 128], [1, 4096]])),
                                    f"store:k{k2}")
                        elif g2 % 2 == 1:
                            # tail: stream the final chunk out in quarters
                            lab(nc.sync.dma_start(
                                out=bass.AP(y, k2 * 524288 + (g2 - 1) * 512,
                                            [[4096, 128], [1, 1024]]),
                                in_=bass.AP(stg_t, stg_o + (g2 - 1) * 512,
                                            [[4096, 128], [1, 1024]])),
                                f"store:k{k2}.{g2}")

    nc.finalize()
    return nc


def _build_waug(w: np.ndarray, kind: str) -> np.ndarray:
    """Augmented 128x128 weights (see apsim2.py)."""
    wa = np.zeros((128, 128), dtype=np.float32)
    ar = np.arange(32)
    if kind == "w3":
        # rows p = j3*4 + b ; cols n = i3a*16 + b*4 + i3b
        for b in range(4):
            cols = (ar >> 2) * 16 + b * 4 + (ar & 3)
            wa[np.ix_(ar * 4 + b, cols)] = w.T
    else:
        # rows p = q*32 + j ; cols n = i*4 + q
        for q in range(4):
            wa[np.ix_(q * 32 + ar, ar * 4 + q)] = w.T
    return wa


def _get_nc():
    if "nc" not in _NC_CACHE:
        _NC_CACHE["nc"] = _build_nc()
    return _NC_CACHE["nc"]


def make_in_maps(x, W0, W1, W2, W3):
    x = np.asarray(x, dtype=np.float32)
    bf = ml_dtypes.bfloat16
    w2a = _build_waug(np.asarray(W2, np.float32), "q").astype(bf)
    w3a = _build_waug(np.asarray(W3, np.float32), "w3").astype(bf)
    w1a = _build_waug(np.asarray(W1, np.float32), "q").astype(bf)
    w0a = _build_waug(np.asarray(W0, np.float32), "q").astype(bf)
    xr = x.reshape(32, 4, 8, 32, 32, B)
    in_maps = []
    for c in range(NCORES):
        xc = xr[..., c * BC:(c + 1) * BC].transpose(0, 1, 3, 2, 4, 5)
        xc = np.ascontiguousarray(xc).astype(bf).reshape(32, 131072)
        in_maps.append({"x": xc, "w2a": w2a, "w3a": w3a,
                        "w1a": w1a, "w0a": w0a})
    return in_maps


def _unshuffle_y(yd: np.ndarray) -> np.ndarray:
    """[i2a(8), (i0, i3b), (i3a, b, i1, i2b)] -> [N, BC]."""
    y = yd.astype(np.float32).reshape(8, 32, 4, 8, BC, 32, 4)
    y = y.transpose(1, 5, 0, 6, 3, 2, 4)
    return np.ascontiguousarray(y).reshape(N, BC)


def kernel(x, W0, W1, W2, W3, _trace=False):
    nc = _get_nc()
    in_maps = make_in_maps(x, W0, W1, W2, W3)
    res = run_bass_kernel_spmd(nc, in_maps, core_ids=list(range(NCORES)),
                               trace=_trace)
    out = np.concatenate(
        [_unshuffle_y(res.results[c]["y"]) for c in range(NCORES)], axis=1)
    if _trace:
        kernel.last_result = res
    return out


if __name__ == "__main__":
    rng = np.random.default_rng(0)
    x = rng.standard_normal((N, B), dtype=np.float32)
    ws = [rng.standard_normal((L, L), dtype=np.float32) for _ in range(4)]
    y = kernel(x, *ws)
    print("ran", y.shape, y.dtype)
